# revision 5
# baseline (speedup 1.0000x reference)
"""Multi-head attention (B=2, S=2048, E=1024, H=16, D=64) on 8 TRN2 cores.

Sharding: core c handles batch b = c//4 and head-group g = c%4 (4 heads,
256 embed cols). No cross-core communication; host slices inputs and
gathers/normalizes outputs.

Per-core device program (all matmuls bf16, accumulation fp32 in PSUM):
  - inputs arrive pre-transposed ([E, S]) so projections need no on-chip
    transpose: qhT[c, s] = sum_e wq[e, c] * qT[e, s]  (c on partitions)
  - scoresT[j, i] = sum_d khT[d, j] * qhT[d, i]  (K=64 contraction)
  - expT = exp(0.125 * scoresT)  via ScalarE, PSUM -> SBUF bf16
  - out_raw[m, i] = sum_j vh_aug[j, m] * expT[j, i]  where vh_aug has a
    ones column at m=64, so row 64 accumulates the softmax denominator.
  - host divides by the denominator, adds the V bias (distributes through
    softmax), and transposes into the final [B, S, E] layout.
  - K-projection bias is skipped entirely: it shifts every score row by a
    constant in j, which softmax cancels.
"""

import sys

sys.path.insert(0, "/opt/trn_rl_repo")

import numpy as np

import concourse.bass as bass  # noqa: F401  (registers AP types)
import concourse.mybir as mybir
from concourse import bacc
from concourse.tile import TileContext

B, S, E = 2, 2048, 1024
H, D = 16, 64
HPC = 4  # heads per core
COLS = HPC * D  # 256 embed columns per core
P = 128
F32 = mybir.dt.float32
BF16 = mybir.dt.bfloat16
ET = E // P  # 8 e-tiles
JT = S // P  # 16 j-tiles
NB = 512  # matmul moving free dim
IH = 2  # i halves (1024 each)

_CACHED = {}


def build():
    nc = bacc.Bacc("TRN2", target_bir_lowering=False, debug=False)
    qT = nc.dram_tensor("qT", [E, S], F32, kind="ExternalInput")
    kT = nc.dram_tensor("kT", [E, S], F32, kind="ExternalInput")
    vT = nc.dram_tensor("vT", [E, S], F32, kind="ExternalInput")
    wq = nc.dram_tensor("wq", [E, COLS], F32, kind="ExternalInput")
    wk = nc.dram_tensor("wk", [E, COLS], F32, kind="ExternalInput")
    wv = nc.dram_tensor("wv", [E, COLS], F32, kind="ExternalInput")
    bq = nc.dram_tensor("bq", [P, 2], F32, kind="ExternalInput")
    out_raw = nc.dram_tensor("out_raw", [65, 8 * 1024], F32, kind="ExternalOutput")

    with TileContext(nc) as tc:
        with (
            tc.tile_pool(name="wp", bufs=1) as wp,
            tc.tile_pool(name="xq", bufs=ET) as xq,
            tc.tile_pool(name="xk", bufs=ET) as xk,
            tc.tile_pool(name="xv", bufs=ET) as xv,
            tc.tile_pool(name="hp", bufs=1) as hp,
            tc.tile_pool(name="pe", bufs=3) as pe,
            tc.tile_pool(name="psA", bufs=2, space="PSUM") as psA,
            tc.tile_pool(name="psO", bufs=2, space="PSUM") as psO,
        ):
            # --- weights (cast to bf16 during DMA) + bias ---
            wq_b = wp.tile([P, ET, COLS], BF16)
            wk_b = wp.tile([P, ET, COLS], BF16)
            wv_b = wp.tile([P, ET, COLS], BF16)
            nc.gpsimd.dma_start(wq_b, wq.rearrange("(t p) c -> p t c", p=P))
            nc.gpsimd.dma_start(wk_b, wk.rearrange("(t p) c -> p t c", p=P))
            nc.gpsimd.dma_start(wv_b, wv.rearrange("(t p) c -> p t c", p=P))
            bq_t = wp.tile([P, 2], F32)
            nc.sync.dma_start(bq_t, bq[:, :])

            # --- activations (cast to bf16 during DMA) ---
            def load_x(pool, dram, tag):
                tiles = []
                for et in range(ET):
                    t = pool.tile([P, S], BF16, tag=tag, name=f"{tag}{et}")
                    nc.gpsimd.dma_start(t, dram[et * P : (et + 1) * P, :])
                    tiles.append(t)
                return tiles

            qx = load_x(xq, qT, "qx")
            kx = load_x(xk, kT, "kx")
            vx = load_x(xv, vT, "vx")

            # --- resident head tensors ---
            qhT = hp.tile([P, 2, S], BF16)  # [2 heads * 64 d, chunk, s]
            khT = hp.tile([P, 2, S], BF16)
            vh_aug = hp.tile([P, JT, HPC * 65], BF16)
            out_sb = hp.tile([P, 8, 1024], F32)  # only rows 0..64 used
            nc.vector.memset(vh_aug, 1.0)

            # --- Q/K projections -> transposed head layout [c, s] ---
            for name, w_b, dst, bias in (
                ("q", wq_b, qhT, bq_t),
                ("k", wk_b, khT, None),
            ):
                x = qx if name == "q" else kx
                for ch in range(2):
                    for sb in range(S // NB):
                        ps = psA.tile([P, 1024], F32, tag="s", name=f"ps_{name}")
                        for et in range(ET):
                            nc.tensor.matmul(
                                ps[:, :NB],
                                w_b[:, et, ch * P : (ch + 1) * P],
                                x[et][:, sb * NB : (sb + 1) * NB],
                                start=(et == 0),
                                stop=(et == ET - 1),
                            )
                        if bias is not None:
                            nc.vector.tensor_scalar_add(
                                dst[:, ch, sb * NB : (sb + 1) * NB],
                                ps[:, :NB],
                                bias[:, ch : ch + 1],
                            )
                        else:
                            nc.vector.tensor_copy(
                                dst[:, ch, sb * NB : (sb + 1) * NB], ps[:, :NB]
                            )

            # --- V projection -> natural [s, c] layout, no bias (host adds) ---
            for sc in range(JT):
                ps = psA.tile([P, 1024], F32, tag="s", name="ps_v")
                for et in range(ET):
                    nc.tensor.matmul(
                        ps[:, :COLS],
                        vx[et][:, sc * P : (sc + 1) * P],
                        wv_b[:, et, :],
                        start=(et == 0),
                        stop=(et == ET - 1),
                    )
                nc.vector.tensor_copy(
                    vh_aug[:, sc].rearrange("p (h x) -> p h x", x=65)[:, :, :D],
                    ps[:, :COLS].rearrange("p (h x) -> p h x", x=D),
                )

            # --- attention ---
            for h in range(HPC):
                h2 = (h % 2) * D
                ch = h // 2
                for ih in range(IH):
                    ops = psO.tile([P, 1024], F32, tag="o", name="ops")
                    for jt in range(JT):
                        sps = psA.tile([P, 1024], F32, tag="s", name="sps")
                        for qq in range(2):
                            i0 = ih * 1024 + qq * NB
                            nc.tensor.matmul(
                                sps[:, qq * NB : (qq + 1) * NB],
                                khT[h2 : h2 + D, ch, jt * P : (jt + 1) * P],
                                qhT[h2 : h2 + D, ch, i0 : i0 + NB],
                                start=True,
                                stop=True,
                            )
                        expT = pe.tile([P, 1024], BF16, tag="e", name="expT")
                        nc.scalar.activation(
                            expT, sps, mybir.ActivationFunctionType.Exp, scale=0.125
                        )
                        for qq in range(2):
                            nc.tensor.matmul(
                                ops[:65, qq * NB : (qq + 1) * NB],
                                vh_aug[:, jt, h * 65 : (h + 1) * 65],
                                expT[:, qq * NB : (qq + 1) * NB],
                                start=(jt == 0),
                                stop=(jt == JT - 1),
                            )
                    r = h * IH + ih
                    nc.vector.tensor_copy(out_sb[:65, r, :], ops[:65, :])
                    nc.sync.dma_start(
                        out_raw[:, r * 1024 : (r + 1) * 1024], out_sb[:65, r, :]
                    )
    nc.finalize()
    return nc


def _prep_in_maps(q, k, v, wq, bq, wk, bk, wv, bv):
    q, k, v = (np.asarray(x, np.float32) for x in (q, k, v))
    wq, wk, wv = (np.asarray(x, np.float32) for x in (wq, wk, wv))
    bq = np.asarray(bq, np.float32)
    qT = [np.ascontiguousarray(q[b].T) for b in range(B)]
    kT = [np.ascontiguousarray(k[b].T) for b in range(B)]
    vT = [np.ascontiguousarray(v[b].T) for b in range(B)]
    in_maps = []
    for c in range(8):
        b, g = divmod(c, 4)
        cs = slice(g * COLS, (g + 1) * COLS)
        in_maps.append(
            {
                "qT": qT[b],
                "kT": kT[b],
                "vT": vT[b],
                "wq": np.ascontiguousarray(wq[:, cs]),
                "wk": np.ascontiguousarray(wk[:, cs]),
                "wv": np.ascontiguousarray(wv[:, cs]),
                "bq": np.ascontiguousarray(bq[cs].reshape(2, P).T),
            }
        )
    return in_maps


def _make_runner(nc, n_cores=8):
    """Persistent jitted shard_map runner over the prebuilt Bass module.

    Replicates concourse.bass2jax.run_bass_via_pjrt's multi-core path but
    caches the jitted callable (and the zero output buffers) so repeat
    kernel() calls skip retracing, and timing loops can rerun the NEFF
    without re-transferring anything but the inputs.
    """
    import jax
    from jax.experimental.shard_map import shard_map
    from jax.sharding import Mesh, PartitionSpec
    from concourse import bass2jax

    bass2jax.install_neuronx_cc_hook()

    in_names, out_names, out_avals, zero_outs = [], [], [], []
    for alloc in nc.m.functions[0].allocations:
        if not isinstance(alloc, mybir.MemoryLocationSet):
            continue
        name = alloc.memorylocations[0].name
        if alloc.kind == "ExternalInput":
            in_names.append(name)
        elif alloc.kind == "ExternalOutput":
            shape = tuple(alloc.tensor_shape)
            dtype = mybir.dt.np(alloc.dtype)
            out_avals.append(jax.core.ShapedArray(shape, dtype))
            zero_outs.append(np.zeros((n_cores * shape[0], *shape[1:]), dtype))
            out_names.append(name)
    n_params = len(in_names)
    all_names = in_names + out_names
    pid_name = nc.partition_id_tensor.name if nc.partition_id_tensor else None
    if pid_name is not None:
        in_names = [n for n in in_names if n != pid_name]
        all_names = in_names + out_names + [pid_name]
        n_params = len(in_names)

    def _body(*args):
        operands = list(args)
        if pid_name is not None:
            operands.append(bass2jax.partition_id_tensor())
        outs = bass2jax._bass_exec_p.bind(
            *operands,
            out_avals=tuple(out_avals),
            in_names=tuple(all_names),
            out_names=tuple(out_names),
            lowering_input_output_aliases=(),
            sim_require_finite=True,
            sim_require_nnan=True,
            nc=nc,
        )
        return tuple(outs)

    devices = jax.devices()[:n_cores]
    mesh = Mesh(np.asarray(devices), ("core",))
    nio = n_params + len(out_names)
    sharded = jax.jit(
        shard_map(
            _body,
            mesh=mesh,
            in_specs=(PartitionSpec("core"),) * nio,
            out_specs=(PartitionSpec("core"),) * len(out_names),
            check_rep=False,
        ),
        keep_unused=True,
    )
    zeros_dev = [jax.device_put(z) for z in zero_outs]

    def run(in_maps):
        concat_in = [
            np.concatenate([np.asarray(m[name]) for m in in_maps], axis=0)
            for name in in_names
        ]
        out_arrs = sharded(*concat_in, *zeros_dev)
        return [
            {
                name: np.asarray(out_arrs[i]).reshape(
                    n_cores, *out_avals[i].shape
                )[c]
                for i, name in enumerate(out_names)
            }
            for c in range(n_cores)
        ]

    run.sharded = sharded
    run.in_names = in_names
    run.zeros_dev = zeros_dev
    return run


def get_runner():
    if "run" not in _CACHED:
        _CACHED["run"] = _make_runner(build())
    return _CACHED["run"]


def kernel(q, k, v, wq, bq, wk, bk, wv, bv):
    run = get_runner()
    in_maps = _prep_in_maps(q, k, v, wq, bq, wk, bk, wv, bv)
    results = run(in_maps)

    bv = np.asarray(bv, np.float32)
    out = np.empty((B, S, E), np.float32)
    for c in range(8):
        b, g = divmod(c, 4)
        raw = results[c]["out_raw"]  # [65, 8192]
        num = raw[:64].reshape(64, HPC, IH * 1024)
        den = raw[64].reshape(HPC, IH * 1024)
        for h in range(HPC):
            col0 = g * COLS + h * D
            o = num[:, h, :] / den[h][None, :]  # [64, 2048]
            out[b, :, col0 : col0 + D] = o.T + bv[col0 : col0 + D][None, :]
    return out


# revision 7
# speedup vs baseline: 18.1541x; 18.1541x over previous
"""Multi-head attention (B=2, S=2048, E=1024, H=16, D=64) on 8 TRN2 cores.

Sharding: core c handles batch b = c//4 and head-group g = c%4 (4 heads,
256 embed cols). No cross-core communication; host slices inputs and
gathers/normalizes outputs.

Per-core device program (all matmuls bf16, accumulation fp32 in PSUM):
  - inputs arrive pre-transposed ([E, S]) so projections need no on-chip
    transpose: qhT[c, s] = sum_e wq[e, c] * qT[e, s]  (c on partitions)
  - scoresT[j, i] = sum_d khT[d, j] * qhT[d, i]  (K=64 contraction)
  - expT = exp(0.125 * scoresT)  via ScalarE, PSUM -> SBUF bf16
  - out_raw[m, i] = sum_j vh_aug[j, m] * expT[j, i]  where vh_aug has a
    ones column at m=64, so row 64 accumulates the softmax denominator.
  - host divides by the denominator, adds the V bias (distributes through
    softmax), and transposes into the final [B, S, E] layout.
  - K-projection bias is skipped entirely: it shifts every score row by a
    constant in j, which softmax cancels.
"""

import sys

sys.path.insert(0, "/opt/trn_rl_repo")

import numpy as np

import concourse.bass as bass  # noqa: F401  (registers AP types)
import concourse.mybir as mybir
from concourse import bacc
from concourse.tile import TileContext

B, S, E = 2, 2048, 1024
H, D = 16, 64
HPC = 4  # heads per core
COLS = HPC * D  # 256 embed columns per core
P = 128
F32 = mybir.dt.float32
BF16 = mybir.dt.bfloat16
ET = E // P  # 8 e-tiles
JT = S // P  # 16 j-tiles
NB = 512  # matmul moving free dim
IH = 2  # i halves (1024 each)

_CACHED = {}


def build():
    nc = bacc.Bacc("TRN2", target_bir_lowering=False, debug=False)
    qT = nc.dram_tensor("qT", [E, S], F32, kind="ExternalInput")
    kT = nc.dram_tensor("kT", [E, S], F32, kind="ExternalInput")
    vT = nc.dram_tensor("vT", [E, S], F32, kind="ExternalInput")
    wq = nc.dram_tensor("wq", [E, COLS], F32, kind="ExternalInput")
    wk = nc.dram_tensor("wk", [E, COLS], F32, kind="ExternalInput")
    wv = nc.dram_tensor("wv", [E, COLS], F32, kind="ExternalInput")
    bq = nc.dram_tensor("bq", [P, 2], F32, kind="ExternalInput")
    out_raw = nc.dram_tensor("out_raw", [65, 8 * 1024], F32, kind="ExternalOutput")

    with TileContext(nc) as tc:
        with (
            tc.tile_pool(name="wp", bufs=1) as wp,
            tc.tile_pool(name="xq", bufs=ET) as xq,
            tc.tile_pool(name="xk", bufs=ET) as xk,
            tc.tile_pool(name="xv", bufs=ET) as xv,
            tc.tile_pool(name="hp", bufs=1) as hp,
            tc.tile_pool(name="pe", bufs=3) as pe,
            tc.tile_pool(name="psA", bufs=2, space="PSUM") as psA,
            tc.tile_pool(name="psO", bufs=2, space="PSUM") as psO,
        ):
            # --- weights (cast to bf16 during DMA) + bias ---
            wq_b = wp.tile([P, ET, COLS], BF16)
            wk_b = wp.tile([P, ET, COLS], BF16)
            wv_b = wp.tile([P, ET, COLS], BF16)
            nc.gpsimd.dma_start(wq_b, wq.rearrange("(t p) c -> p t c", p=P))
            nc.gpsimd.dma_start(wk_b, wk.rearrange("(t p) c -> p t c", p=P))
            nc.gpsimd.dma_start(wv_b, wv.rearrange("(t p) c -> p t c", p=P))
            bq_t = wp.tile([P, 2], F32)
            nc.sync.dma_start(bq_t, bq[:, :])

            # --- activations (cast to bf16 during DMA) ---
            def load_x(pool, dram, tag):
                tiles = []
                for et in range(ET):
                    t = pool.tile([P, S], BF16, tag=tag, name=f"{tag}{et}")
                    nc.gpsimd.dma_start(t, dram[et * P : (et + 1) * P, :])
                    tiles.append(t)
                return tiles

            qx = load_x(xq, qT, "qx")
            kx = load_x(xk, kT, "kx")
            vx = load_x(xv, vT, "vx")

            # --- resident head tensors ---
            qhT = hp.tile([P, 2, S], BF16)  # [2 heads * 64 d, chunk, s]
            khT = hp.tile([P, 2, S], BF16)
            vh_aug = hp.tile([P, JT, HPC * 65], BF16)
            out_sb = hp.tile([P, 8, 1024], F32)  # only rows 0..64 used
            nc.vector.memset(vh_aug, 1.0)

            # --- Q/K projections -> transposed head layout [c, s] ---
            for name, w_b, dst, bias in (
                ("q", wq_b, qhT, bq_t),
                ("k", wk_b, khT, None),
            ):
                x = qx if name == "q" else kx
                for ch in range(2):
                    for sb in range(S // NB):
                        ps = psA.tile([P, 1024], F32, tag="s", name=f"ps_{name}")
                        for et in range(ET):
                            nc.tensor.matmul(
                                ps[:, :NB],
                                w_b[:, et, ch * P : (ch + 1) * P],
                                x[et][:, sb * NB : (sb + 1) * NB],
                                start=(et == 0),
                                stop=(et == ET - 1),
                            )
                        if bias is not None:
                            nc.vector.tensor_scalar_add(
                                dst[:, ch, sb * NB : (sb + 1) * NB],
                                ps[:, :NB],
                                bias[:, ch : ch + 1],
                            )
                        else:
                            nc.vector.tensor_copy(
                                dst[:, ch, sb * NB : (sb + 1) * NB], ps[:, :NB]
                            )

            # --- V projection -> natural [s, c] layout, no bias (host adds) ---
            for sc in range(JT):
                ps = psA.tile([P, 1024], F32, tag="s", name="ps_v")
                for et in range(ET):
                    nc.tensor.matmul(
                        ps[:, :COLS],
                        vx[et][:, sc * P : (sc + 1) * P],
                        wv_b[:, et, :],
                        start=(et == 0),
                        stop=(et == ET - 1),
                    )
                nc.vector.tensor_copy(
                    vh_aug[:, sc].rearrange("p (h x) -> p h x", x=65)[:, :, :D],
                    ps[:, :COLS].rearrange("p (h x) -> p h x", x=D),
                )

            # --- attention ---
            for h in range(HPC):
                h2 = (h % 2) * D
                ch = h // 2
                for ih in range(IH):
                    ops = psO.tile([P, 1024], F32, tag="o", name="ops")
                    for jt in range(JT):
                        sps = psA.tile([P, 1024], F32, tag="s", name="sps")
                        for qq in range(2):
                            i0 = ih * 1024 + qq * NB
                            nc.tensor.matmul(
                                sps[:, qq * NB : (qq + 1) * NB],
                                khT[h2 : h2 + D, ch, jt * P : (jt + 1) * P],
                                qhT[h2 : h2 + D, ch, i0 : i0 + NB],
                                start=True,
                                stop=True,
                            )
                        expT = pe.tile([P, 1024], BF16, tag="e", name="expT")
                        nc.scalar.activation(
                            expT, sps, mybir.ActivationFunctionType.Exp, scale=0.125
                        )
                        for qq in range(2):
                            nc.tensor.matmul(
                                ops[:65, qq * NB : (qq + 1) * NB],
                                vh_aug[:, jt, h * 65 : (h + 1) * 65],
                                expT[:, qq * NB : (qq + 1) * NB],
                                start=(jt == 0),
                                stop=(jt == JT - 1),
                            )
                    r = h * IH + ih
                    nc.vector.tensor_copy(out_sb[:65, r, :], ops[:65, :])
                    nc.sync.dma_start(
                        out_raw[:, r * 1024 : (r + 1) * 1024], out_sb[:65, r, :]
                    )
    nc.finalize()
    return nc


def _prep_in_maps(q, k, v, wq, bq, wk, bk, wv, bv):
    q, k, v = (np.asarray(x, np.float32) for x in (q, k, v))
    wq, wk, wv = (np.asarray(x, np.float32) for x in (wq, wk, wv))
    bq = np.asarray(bq, np.float32)
    qT = [np.ascontiguousarray(q[b].T) for b in range(B)]
    kT = [np.ascontiguousarray(k[b].T) for b in range(B)]
    vT = [np.ascontiguousarray(v[b].T) for b in range(B)]
    in_maps = []
    for c in range(8):
        b, g = divmod(c, 4)
        cs = slice(g * COLS, (g + 1) * COLS)
        in_maps.append(
            {
                "qT": qT[b],
                "kT": kT[b],
                "vT": vT[b],
                "wq": np.ascontiguousarray(wq[:, cs]),
                "wk": np.ascontiguousarray(wk[:, cs]),
                "wv": np.ascontiguousarray(wv[:, cs]),
                "bq": np.ascontiguousarray(bq[cs].reshape(2, P).T),
            }
        )
    return in_maps


def _make_runner(nc, n_cores=8):
    """Persistent jitted shard_map runner over the prebuilt Bass module.

    Replicates concourse.bass2jax.run_bass_via_pjrt's multi-core path but
    caches the jitted callable (and the zero output buffers) so repeat
    kernel() calls skip retracing, and timing loops can rerun the NEFF
    without re-transferring anything but the inputs.
    """
    import jax
    from jax.experimental.shard_map import shard_map
    from jax.sharding import Mesh, PartitionSpec
    from concourse import bass2jax

    bass2jax.install_neuronx_cc_hook()

    in_names, out_names, out_avals, zero_outs = [], [], [], []
    for alloc in nc.m.functions[0].allocations:
        if not isinstance(alloc, mybir.MemoryLocationSet):
            continue
        name = alloc.memorylocations[0].name
        if alloc.kind == "ExternalInput":
            in_names.append(name)
        elif alloc.kind == "ExternalOutput":
            shape = tuple(alloc.tensor_shape)
            dtype = mybir.dt.np(alloc.dtype)
            out_avals.append(jax.core.ShapedArray(shape, dtype))
            zero_outs.append(np.zeros((n_cores * shape[0], *shape[1:]), dtype))
            out_names.append(name)
    n_params = len(in_names)
    all_names = in_names + out_names
    pid_name = nc.partition_id_tensor.name if nc.partition_id_tensor else None
    if pid_name is not None:
        in_names = [n for n in in_names if n != pid_name]
        all_names = in_names + out_names + [pid_name]
        n_params = len(in_names)

    def _body(*args):
        operands = list(args)
        if pid_name is not None:
            operands.append(bass2jax.partition_id_tensor())
        outs = bass2jax._bass_exec_p.bind(
            *operands,
            out_avals=tuple(out_avals),
            in_names=tuple(all_names),
            out_names=tuple(out_names),
            lowering_input_output_aliases=(),
            sim_require_finite=True,
            sim_require_nnan=True,
            nc=nc,
        )
        return tuple(outs)

    devices = jax.devices()[:n_cores]
    mesh = Mesh(np.asarray(devices), ("core",))
    nio = n_params + len(out_names)
    sharded = jax.jit(
        shard_map(
            _body,
            mesh=mesh,
            in_specs=(PartitionSpec("core"),) * nio,
            out_specs=(PartitionSpec("core"),) * len(out_names),
            check_rep=False,
        ),
        keep_unused=True,
    )
    from jax.sharding import NamedSharding

    row_sharding = NamedSharding(mesh, PartitionSpec("core"))
    zeros_dev = [jax.device_put(z, row_sharding) for z in zero_outs]

    def run(in_maps):
        concat_in = [
            np.concatenate([np.asarray(m[name]) for m in in_maps], axis=0)
            for name in in_names
        ]
        out_arrs = sharded(*concat_in, *zeros_dev)
        return [
            {
                name: np.asarray(out_arrs[i]).reshape(
                    n_cores, *out_avals[i].shape
                )[c]
                for i, name in enumerate(out_names)
            }
            for c in range(n_cores)
        ]

    run.sharded = sharded
    run.in_names = in_names
    run.zeros_dev = zeros_dev
    run.row_sharding = row_sharding
    return run


def get_runner():
    if "run" not in _CACHED:
        _CACHED["run"] = _make_runner(build())
    return _CACHED["run"]


def kernel(q, k, v, wq, bq, wk, bk, wv, bv):
    run = get_runner()
    in_maps = _prep_in_maps(q, k, v, wq, bq, wk, bk, wv, bv)
    results = run(in_maps)

    bv = np.asarray(bv, np.float32)
    out = np.empty((B, S, E), np.float32)
    for c in range(8):
        b, g = divmod(c, 4)
        raw = results[c]["out_raw"]  # [65, 8192]
        num = raw[:64].reshape(64, HPC, IH * 1024)
        den = raw[64].reshape(HPC, IH * 1024)
        for h in range(HPC):
            col0 = g * COLS + h * D
            o = num[:, h, :] / den[h][None, :]  # [64, 2048]
            out[b, :, col0 : col0 + D] = o.T + bv[col0 : col0 + D][None, :]
    return out


# revision 8
# speedup vs baseline: 73.9607x; 4.0741x over previous
"""Multi-head attention (B=2, S=2048, E=1024, H=16, D=64) on 8 TRN2 cores.

Sharding: core c handles batch b = c//4 and head-group g = c%4 (4 heads,
256 embed cols). No cross-core communication; host slices inputs and
gathers/normalizes outputs.

Per-core device program (all matmuls bf16, accumulation fp32 in PSUM):
  - inputs arrive pre-transposed ([E, S]) so projections need no on-chip
    transpose: qhT[c, s] = sum_e wq[e, c] * qT[e, s]  (c on partitions)
  - scoresT[j, i] = sum_d khT[d, j] * qhT[d, i]  (K=64 contraction)
  - expT = exp(0.125 * scoresT)  via ScalarE, PSUM -> SBUF bf16
  - out_raw[m, i] = sum_j vh_aug[j, m] * expT[j, i]  where vh_aug has a
    ones column at m=64, so row 64 accumulates the softmax denominator.
  - host divides by the denominator, adds the V bias (distributes through
    softmax), and transposes into the final [B, S, E] layout.
  - K-projection bias is skipped entirely: it shifts every score row by a
    constant in j, which softmax cancels.
"""

import sys

sys.path.insert(0, "/opt/trn_rl_repo")

import numpy as np

import concourse.bass as bass  # noqa: F401  (registers AP types)
import concourse.mybir as mybir
from concourse import bacc
from concourse.tile import TileContext

B, S, E = 2, 2048, 1024
H, D = 16, 64
HPC = 4  # heads per core
COLS = HPC * D  # 256 embed columns per core
P = 128
F32 = mybir.dt.float32
BF16 = mybir.dt.bfloat16
ET = E // P  # 8 e-tiles
JT = S // P  # 16 j-tiles
NB = 512  # matmul moving free dim
IH = 2  # i halves (1024 each)

_CACHED = {}


def build():
    nc = bacc.Bacc("TRN2", target_bir_lowering=False, debug=False)
    qT = nc.dram_tensor("qT", [E, S], F32, kind="ExternalInput")
    kT = nc.dram_tensor("kT", [E, S], F32, kind="ExternalInput")
    vT = nc.dram_tensor("vT", [E, S], F32, kind="ExternalInput")
    wq = nc.dram_tensor("wq", [E, COLS], F32, kind="ExternalInput")
    wk = nc.dram_tensor("wk", [E, COLS], F32, kind="ExternalInput")
    wv = nc.dram_tensor("wv", [E, COLS], F32, kind="ExternalInput")
    bq = nc.dram_tensor("bq", [P, 2], F32, kind="ExternalInput")
    out_raw = nc.dram_tensor("out_raw", [65, 8 * 1024], F32, kind="ExternalOutput")

    with TileContext(nc) as tc:
        with (
            tc.tile_pool(name="wp", bufs=1) as wp,
            tc.tile_pool(name="xq", bufs=ET) as xq,
            tc.tile_pool(name="xk", bufs=ET) as xk,
            tc.tile_pool(name="xv", bufs=ET) as xv,
            tc.tile_pool(name="hp", bufs=1) as hp,
            tc.tile_pool(name="pe", bufs=3) as pe,
            tc.tile_pool(name="psA", bufs=2, space="PSUM") as psA,
            tc.tile_pool(name="psO", bufs=2, space="PSUM") as psO,
        ):
            # --- weights (cast to bf16 during DMA) + bias ---
            wq_b = wp.tile([P, ET, COLS], BF16)
            wk_b = wp.tile([P, ET, COLS], BF16)
            wv_b = wp.tile([P, ET, COLS], BF16)
            nc.gpsimd.dma_start(wq_b, wq.rearrange("(t p) c -> p t c", p=P))
            nc.gpsimd.dma_start(wk_b, wk.rearrange("(t p) c -> p t c", p=P))
            nc.gpsimd.dma_start(wv_b, wv.rearrange("(t p) c -> p t c", p=P))
            bq_t = wp.tile([P, 2], F32)
            nc.sync.dma_start(bq_t, bq[:, :])

            # --- activations (cast to bf16 during DMA) ---
            def load_x(pool, dram, tag):
                tiles = []
                for et in range(ET):
                    t = pool.tile([P, S], BF16, tag=tag, name=f"{tag}{et}")
                    nc.gpsimd.dma_start(t, dram[et * P : (et + 1) * P, :])
                    tiles.append(t)
                return tiles

            qx = load_x(xq, qT, "qx")
            kx = load_x(xk, kT, "kx")
            vx = load_x(xv, vT, "vx")

            # --- resident head tensors ---
            qhT = hp.tile([P, 2, S], BF16)  # [2 heads * 64 d, chunk, s]
            khT = hp.tile([P, 2, S], BF16)
            vh_aug = hp.tile([P, JT, HPC * 65], BF16)
            out_sb = hp.tile([P, 8, 1024], F32)  # only rows 0..64 used
            nc.vector.memset(vh_aug, 1.0)

            # --- Q/K projections -> transposed head layout [c, s] ---
            for name, w_b, dst, bias in (
                ("q", wq_b, qhT, bq_t),
                ("k", wk_b, khT, None),
            ):
                x = qx if name == "q" else kx
                for ch in range(2):
                    for sb in range(S // NB):
                        ps = psA.tile([P, 1024], F32, tag="s", name=f"ps_{name}")
                        for et in range(ET):
                            nc.tensor.matmul(
                                ps[:, :NB],
                                w_b[:, et, ch * P : (ch + 1) * P],
                                x[et][:, sb * NB : (sb + 1) * NB],
                                start=(et == 0),
                                stop=(et == ET - 1),
                            )
                        if bias is not None:
                            nc.vector.tensor_scalar_add(
                                dst[:, ch, sb * NB : (sb + 1) * NB],
                                ps[:, :NB],
                                bias[:, ch : ch + 1],
                            )
                        else:
                            nc.vector.tensor_copy(
                                dst[:, ch, sb * NB : (sb + 1) * NB], ps[:, :NB]
                            )

            # --- V projection -> natural [s, c] layout, no bias (host adds) ---
            for sc in range(JT):
                ps = psA.tile([P, 1024], F32, tag="s", name="ps_v")
                for et in range(ET):
                    nc.tensor.matmul(
                        ps[:, :COLS],
                        vx[et][:, sc * P : (sc + 1) * P],
                        wv_b[:, et, :],
                        start=(et == 0),
                        stop=(et == ET - 1),
                    )
                nc.vector.tensor_copy(
                    vh_aug[:, sc].rearrange("p (h x) -> p h x", x=65)[:, :, :D],
                    ps[:, :COLS].rearrange("p (h x) -> p h x", x=D),
                )

            # --- attention ---
            for h in range(HPC):
                h2 = (h % 2) * D
                ch = h // 2
                for ih in range(IH):
                    ops = psO.tile([P, 1024], F32, tag="o", name="ops")
                    for jt in range(JT):
                        sps = psA.tile([P, 1024], F32, tag="s", name="sps")
                        for qq in range(2):
                            i0 = ih * 1024 + qq * NB
                            nc.tensor.matmul(
                                sps[:, qq * NB : (qq + 1) * NB],
                                khT[h2 : h2 + D, ch, jt * P : (jt + 1) * P],
                                qhT[h2 : h2 + D, ch, i0 : i0 + NB],
                                start=True,
                                stop=True,
                            )
                        expT = pe.tile([P, 1024], BF16, tag="e", name="expT")
                        nc.scalar.activation(
                            expT, sps, mybir.ActivationFunctionType.Exp, scale=0.125
                        )
                        for qq in range(2):
                            nc.tensor.matmul(
                                ops[:65, qq * NB : (qq + 1) * NB],
                                vh_aug[:, jt, h * 65 : (h + 1) * 65],
                                expT[:, qq * NB : (qq + 1) * NB],
                                start=(jt == 0),
                                stop=(jt == JT - 1),
                            )
                    r = h * IH + ih
                    nc.vector.tensor_copy(out_sb[:65, r, :], ops[:65, :])
                    nc.sync.dma_start(
                        out_raw[:, r * 1024 : (r + 1) * 1024], out_sb[:65, r, :]
                    )
    nc.finalize()
    return nc


def _prep_in_maps(q, k, v, wq, bq, wk, bk, wv, bv):
    q, k, v = (np.asarray(x, np.float32) for x in (q, k, v))
    wq, wk, wv = (np.asarray(x, np.float32) for x in (wq, wk, wv))
    bq = np.asarray(bq, np.float32)
    qT = [np.ascontiguousarray(q[b].T) for b in range(B)]
    kT = [np.ascontiguousarray(k[b].T) for b in range(B)]
    vT = [np.ascontiguousarray(v[b].T) for b in range(B)]
    in_maps = []
    for c in range(8):
        b, g = divmod(c, 4)
        cs = slice(g * COLS, (g + 1) * COLS)
        in_maps.append(
            {
                "qT": qT[b],
                "kT": kT[b],
                "vT": vT[b],
                "wq": np.ascontiguousarray(wq[:, cs]),
                "wk": np.ascontiguousarray(wk[:, cs]),
                "wv": np.ascontiguousarray(wv[:, cs]),
                "bq": np.ascontiguousarray(bq[cs].reshape(2, P).T),
            }
        )
    return in_maps


def _make_runner(nc, n_cores=8):
    """Persistent jitted shard_map runner over the prebuilt Bass module.

    Replicates concourse.bass2jax.run_bass_via_pjrt's multi-core path but
    caches the jitted callable (and the zero output buffers) so repeat
    kernel() calls skip retracing, and timing loops can rerun the NEFF
    without re-transferring anything but the inputs.
    """
    import jax
    from jax.experimental.shard_map import shard_map
    from jax.sharding import Mesh, PartitionSpec
    from concourse import bass2jax

    bass2jax.install_neuronx_cc_hook()

    in_names, out_names, out_avals, zero_outs = [], [], [], []
    for alloc in nc.m.functions[0].allocations:
        if not isinstance(alloc, mybir.MemoryLocationSet):
            continue
        name = alloc.memorylocations[0].name
        if alloc.kind == "ExternalInput":
            in_names.append(name)
        elif alloc.kind == "ExternalOutput":
            shape = tuple(alloc.tensor_shape)
            dtype = mybir.dt.np(alloc.dtype)
            out_avals.append(jax.core.ShapedArray(shape, dtype))
            zero_outs.append(np.zeros((n_cores * shape[0], *shape[1:]), dtype))
            out_names.append(name)
    n_params = len(in_names)
    all_names = in_names + out_names
    pid_name = nc.partition_id_tensor.name if nc.partition_id_tensor else None
    if pid_name is not None:
        in_names = [n for n in in_names if n != pid_name]
        all_names = in_names + out_names + [pid_name]
        n_params = len(in_names)

    def _body(*args):
        operands = list(args)
        if pid_name is not None:
            operands.append(bass2jax.partition_id_tensor())
        outs = bass2jax._bass_exec_p.bind(
            *operands,
            out_avals=tuple(out_avals),
            in_names=tuple(all_names),
            out_names=tuple(out_names),
            lowering_input_output_aliases=(),
            sim_require_finite=True,
            sim_require_nnan=True,
            nc=nc,
        )
        return tuple(outs)

    devices = jax.devices()[:n_cores]
    mesh = Mesh(np.asarray(devices), ("core",))
    nio = n_params + len(out_names)
    sharded = jax.jit(
        shard_map(
            _body,
            mesh=mesh,
            in_specs=(PartitionSpec("core"),) * nio,
            out_specs=(PartitionSpec("core"),) * len(out_names),
            check_rep=False,
        ),
        keep_unused=True,
    )
    from jax.sharding import NamedSharding

    row_sharding = NamedSharding(mesh, PartitionSpec("core"))
    zeros_dev = [jax.device_put(z, row_sharding) for z in zero_outs]

    def run(in_maps):
        concat_in = [
            np.concatenate([np.asarray(m[name]) for m in in_maps], axis=0)
            for name in in_names
        ]
        out_arrs = sharded(*concat_in, *zeros_dev)
        return [
            {
                name: np.asarray(out_arrs[i]).reshape(
                    n_cores, *out_avals[i].shape
                )[c]
                for i, name in enumerate(out_names)
            }
            for c in range(n_cores)
        ]

    run.sharded = sharded
    run.in_names = in_names
    run.zeros_dev = zeros_dev
    run.row_sharding = row_sharding
    return run


def get_runner():
    if "run" not in _CACHED:
        _CACHED["nc"] = build()
        _CACHED["run"] = _make_runner(_CACHED["nc"])
    return _CACHED["run"]


def kernel(q, k, v, wq, bq, wk, bk, wv, bv):
    run = get_runner()
    in_maps = _prep_in_maps(q, k, v, wq, bq, wk, bk, wv, bv)
    results = run(in_maps)

    bv = np.asarray(bv, np.float32)
    out = np.empty((B, S, E), np.float32)
    for c in range(8):
        b, g = divmod(c, 4)
        raw = results[c]["out_raw"]  # [65, 8192]
        num = raw[:64].reshape(64, HPC, IH * 1024)
        den = raw[64].reshape(HPC, IH * 1024)
        for h in range(HPC):
            col0 = g * COLS + h * D
            o = num[:, h, :] / den[h][None, :]  # [64, 2048]
            out[b, :, col0 : col0 + D] = o.T + bv[col0 : col0 + D][None, :]
    return out


# revision 10
# speedup vs baseline: 89.9932x; 1.2168x over previous
"""Multi-head attention (B=2, S=2048, E=1024, H=16, D=64) on 8 TRN2 cores.

Sharding: core c handles batch b = c//4 and head-group g = c%4 (4 heads,
256 embed cols). No cross-core communication; host slices inputs (pre-
transposed and pre-cast to bf16) and gathers/normalizes outputs.

Per-core device program (bf16 matmuls, fp32 PSUM accumulation):
  - projections: qhT[c, s] = sum_e wq[e, c] qT[e, s] (c on partitions), so
    attention needs no on-chip transposes; K-bias dropped (softmax-invariant),
    V-bias applied on host (distributes through softmax).
  - attention processes head PAIRS: the two scores matmuls run concurrently
    on disjoint PE row groups (K=64 each) into one [128, 1024] PSUM tile;
    one ScalarE exp (scale=1/8 fused) covers both heads per (iq, jt).
  - vh carries a ones column (m=64), so the out-stage accumulates the
    softmax denominator in PSUM row 64; host divides.
"""

import sys

sys.path.insert(0, "/opt/trn_rl_repo")

import numpy as np
import ml_dtypes

import concourse.bass as bass  # noqa: F401
import concourse.mybir as mybir
from concourse import bacc
from concourse.tile import TileContext

B, S, E = 2, 2048, 1024
H, D = 16, 64
HPC = 4  # heads per core
COLS = HPC * D  # 256
P = 128
F32 = mybir.dt.float32
BF16 = mybir.dt.bfloat16
ET = E // P  # 8 e-tiles
JT = S // P  # 16 j-tiles
NB = 512
NIQ = S // NB  # 4 i-quarters

_CACHED = {}


def build():
    nc = bacc.Bacc("TRN2", target_bir_lowering=False, debug=False)
    qT = nc.dram_tensor("qT", [E, S], BF16, kind="ExternalInput")
    kT = nc.dram_tensor("kT", [E, S], BF16, kind="ExternalInput")
    vT = nc.dram_tensor("vT", [E, S], BF16, kind="ExternalInput")
    wq = nc.dram_tensor("wq", [E, COLS], BF16, kind="ExternalInput")
    wk = nc.dram_tensor("wk", [E, COLS], BF16, kind="ExternalInput")
    wv = nc.dram_tensor("wv", [E, COLS], BF16, kind="ExternalInput")
    bq = nc.dram_tensor("bq", [P, 2], F32, kind="ExternalInput")
    # out_raw[:, (h*NIQ+iq)*NB : ...]: rows 0-63 numerator (d), row 64 denom
    out_raw = nc.dram_tensor("out_raw", [65, HPC * S], F32,
                             kind="ExternalOutput")  # [65, 8192]

    with TileContext(nc) as tc:
        with (
            tc.tile_pool(name="wp", bufs=1) as wp,
            tc.tile_pool(name="xq", bufs=ET) as xq,
            tc.tile_pool(name="xk", bufs=ET) as xk,
            tc.tile_pool(name="xv", bufs=ET) as xv,
            tc.tile_pool(name="hp", bufs=1) as hp,
            tc.tile_pool(name="pe", bufs=3) as pe,
            tc.tile_pool(name="psA", bufs=2, space="PSUM") as psA,
            tc.tile_pool(name="psO", bufs=4, space="PSUM") as psO,
        ):
            # --- weights + bias (tiny, first so projections never stall) ---
            wq_b = wp.tile([P, ET, COLS], BF16)
            wk_b = wp.tile([P, ET, COLS], BF16)
            wv_b = wp.tile([P, ET, COLS], BF16)
            nc.sync.dma_start(wq_b, wq.rearrange("(t p) c -> p t c", p=P))
            nc.sync.dma_start(wk_b, wk.rearrange("(t p) c -> p t c", p=P))
            nc.sync.dma_start(wv_b, wv.rearrange("(t p) c -> p t c", p=P))
            bq_t = wp.tile([P, 2], F32)
            nc.sync.dma_start(bq_t, bq[:, :])

            # --- activations, q/k interleaved first, v last ---
            def load_x(pool, dram, tag):
                tiles = []
                for et in range(ET):
                    t = pool.tile([P, S], BF16, tag=tag, name=f"{tag}{et}")
                    tiles.append(t)
                return tiles

            qx = load_x(xq, qT, "qx")
            kx = load_x(xk, kT, "kx")
            vx = load_x(xv, vT, "vx")
            for et in range(ET):
                nc.sync.dma_start(qx[et], qT[et * P : (et + 1) * P, :])
                nc.sync.dma_start(kx[et], kT[et * P : (et + 1) * P, :])
            for et in range(ET):
                nc.sync.dma_start(vx[et], vT[et * P : (et + 1) * P, :])

            # --- resident head tensors ---
            qhT = hp.tile([P, 2, S], BF16)  # [2 heads x 64 d, chunk, s]
            khT = hp.tile([P, 2, S], BF16)
            vh_aug = hp.tile([P, JT, HPC * 65], BF16)
            out_sb = hp.tile([P, H // HPC * NIQ * 2, NB], F32)  # [65used, 16, 512]
            nc.vector.memset(vh_aug, 1.0)

            # --- Q/K projections -> transposed head layout [c, s] ---
            for name, w_b, dst, bias in (
                ("q", wq_b, qhT, bq_t),
                ("k", wk_b, khT, None),
            ):
                x = qx if name == "q" else kx
                for sb in range(S // NB):
                    for ch in range(2):
                        ps = psO.tile([P, NB], F32, tag="o", name=f"ps_{name}")
                        for et in range(ET):
                            nc.tensor.matmul(
                                ps,
                                w_b[:, et, ch * P : (ch + 1) * P],
                                x[et][:, sb * NB : (sb + 1) * NB],
                                start=(et == 0),
                                stop=(et == ET - 1),
                            )
                        if bias is not None:
                            nc.vector.tensor_scalar_add(
                                dst[:, ch, sb * NB : (sb + 1) * NB],
                                ps,
                                bias[:, ch : ch + 1],
                            )
                        else:
                            nc.vector.tensor_copy(
                                dst[:, ch, sb * NB : (sb + 1) * NB], ps
                            )

            # --- V projection -> natural [s, c] layout (no bias) ---
            for sc in range(JT):
                ps = psO.tile([P, NB], F32, tag="o", name="ps_v")
                for et in range(ET):
                    nc.tensor.matmul(
                        ps[:, :COLS],
                        vx[et][:, sc * P : (sc + 1) * P],
                        wv_b[:, et, :],
                        start=(et == 0),
                        stop=(et == ET - 1),
                    )
                nc.vector.tensor_copy(
                    vh_aug[:, sc].rearrange("p (h x) -> p h x", x=65)[:, :, :D],
                    ps[:, :COLS].rearrange("p (h x) -> p h x", x=D),
                )

            # --- attention, head pairs ---
            for pr in range(2):  # heads (2pr, 2pr+1) = chunk pr
                for iq in range(NIQ):
                    op0 = psO.tile([P, NB], F32, tag="o", name="op0")
                    op1 = psO.tile([P, NB], F32, tag="o", name="op1")
                    for jt in range(JT):
                        sps = psA.tile([P, 1024], F32, tag="s", name="sps")
                        for hh in range(2):  # row-group packed, concurrent
                            r0 = hh * D
                            nc.tensor.matmul(
                                sps[:, hh * NB : (hh + 1) * NB],
                                khT[r0 : r0 + D, pr, jt * P : (jt + 1) * P],
                                qhT[r0 : r0 + D, pr, iq * NB : (iq + 1) * NB],
                                start=True,
                                stop=True,
                            )
                        expT = pe.tile([P, 1024], BF16, tag="e", name="expT")
                        nc.scalar.activation(
                            expT, sps, mybir.ActivationFunctionType.Exp, scale=0.125
                        )
                        for hh, op in ((0, op0), (1, op1)):
                            h = 2 * pr + hh
                            nc.tensor.matmul(
                                op[:65, :],
                                vh_aug[:, jt, h * 65 : (h + 1) * 65],
                                expT[:, hh * NB : (hh + 1) * NB],
                                start=(jt == 0),
                                stop=(jt == JT - 1),
                            )
                    for hh, op in ((0, op0), (1, op1)):
                        h = 2 * pr + hh
                        r = h * NIQ + iq
                        nc.vector.tensor_copy(out_sb[:65, r, :], op[:65, :])
                        nc.sync.dma_start(
                            out_raw[:, r * NB : (r + 1) * NB], out_sb[:65, r, :]
                        )
    nc.finalize()
    return nc


def _prep_in_maps(q, k, v, wq, bq, wk, bk, wv, bv):
    bf = ml_dtypes.bfloat16
    q, k, v = (np.asarray(x, np.float32) for x in (q, k, v))
    wqb, wkb, wvb = (np.asarray(x, bf) for x in (wq, wk, wv))
    bq = np.asarray(bq, np.float32)
    qT = [np.ascontiguousarray(q[b].T.astype(bf)) for b in range(B)]
    kT = [np.ascontiguousarray(k[b].T.astype(bf)) for b in range(B)]
    vT = [np.ascontiguousarray(v[b].T.astype(bf)) for b in range(B)]
    in_maps = []
    for c in range(8):
        b, g = divmod(c, 4)
        cs = slice(g * COLS, (g + 1) * COLS)
        in_maps.append(
            {
                "qT": qT[b],
                "kT": kT[b],
                "vT": vT[b],
                "wq": np.ascontiguousarray(wqb[:, cs]),
                "wk": np.ascontiguousarray(wkb[:, cs]),
                "wv": np.ascontiguousarray(wvb[:, cs]),
                "bq": np.ascontiguousarray(bq[cs].reshape(2, P).T),
            }
        )
    return in_maps


def _make_runner(nc, n_cores=8):
    """Persistent jitted shard_map runner over the prebuilt Bass module."""
    import jax
    from jax.experimental.shard_map import shard_map
    from jax.sharding import Mesh, NamedSharding, PartitionSpec
    from concourse import bass2jax

    bass2jax.install_neuronx_cc_hook()

    in_names, out_names, out_avals, zero_outs = [], [], [], []
    for alloc in nc.m.functions[0].allocations:
        if not isinstance(alloc, mybir.MemoryLocationSet):
            continue
        name = alloc.memorylocations[0].name
        if alloc.kind == "ExternalInput":
            in_names.append(name)
        elif alloc.kind == "ExternalOutput":
            shape = tuple(alloc.tensor_shape)
            dtype = mybir.dt.np(alloc.dtype)
            out_avals.append(jax.core.ShapedArray(shape, dtype))
            zero_outs.append(np.zeros((n_cores * shape[0], *shape[1:]), dtype))
            out_names.append(name)
    pid_name = nc.partition_id_tensor.name if nc.partition_id_tensor else None
    if pid_name is not None:
        in_names = [n for n in in_names if n != pid_name]
    n_params = len(in_names)
    all_names = in_names + out_names + ([pid_name] if pid_name else [])

    def _body(*args):
        operands = list(args)
        if pid_name is not None:
            operands.append(bass2jax.partition_id_tensor())
        outs = bass2jax._bass_exec_p.bind(
            *operands,
            out_avals=tuple(out_avals),
            in_names=tuple(all_names),
            out_names=tuple(out_names),
            lowering_input_output_aliases=(),
            sim_require_finite=True,
            sim_require_nnan=True,
            nc=nc,
        )
        return tuple(outs)

    devices = jax.devices()[:n_cores]
    mesh = Mesh(np.asarray(devices), ("core",))
    nio = n_params + len(out_names)
    sharded = jax.jit(
        shard_map(
            _body,
            mesh=mesh,
            in_specs=(PartitionSpec("core"),) * nio,
            out_specs=(PartitionSpec("core"),) * len(out_names),
            check_rep=False,
        ),
        keep_unused=True,
    )
    row_sharding = NamedSharding(mesh, PartitionSpec("core"))
    zeros_dev = [jax.device_put(z, row_sharding) for z in zero_outs]

    def run(in_maps):
        concat_in = [
            np.concatenate([np.asarray(m[name]) for m in in_maps], axis=0)
            for name in in_names
        ]
        out_arrs = sharded(*concat_in, *zeros_dev)
        return [
            {
                name: np.asarray(out_arrs[i]).reshape(n_cores, *out_avals[i].shape)[c]
                for i, name in enumerate(out_names)
            }
            for c in range(n_cores)
        ]

    run.sharded = sharded
    run.in_names = in_names
    run.zeros_dev = zeros_dev
    run.row_sharding = row_sharding
    return run


def get_runner():
    if "run" not in _CACHED:
        _CACHED["nc"] = build()
        _CACHED["run"] = _make_runner(_CACHED["nc"])
    return _CACHED["run"]


def kernel(q, k, v, wq, bq, wk, bk, wv, bv):
    run = get_runner()
    in_maps = _prep_in_maps(q, k, v, wq, bq, wk, bk, wv, bv)
    results = run(in_maps)

    bv = np.asarray(bv, np.float32)
    out = np.empty((B, S, E), np.float32)
    for c in range(8):
        b, g = divmod(c, 4)
        raw = results[c]["out_raw"]  # [65, 8192]
        num = raw[:64].reshape(64, HPC, S)  # [d, h, i] (NIQ*NB = S)
        den = raw[64].reshape(HPC, S)
        for h in range(HPC):
            col0 = g * COLS + h * D
            o = num[:, h, :] / den[h][None, :]
            out[b, :, col0 : col0 + D] = o.T + bv[col0 : col0 + D][None, :]
    return out


# revision 12
# speedup vs baseline: 93.7525x; 1.0418x over previous
"""Multi-head attention (B=2, S=2048, E=1024, H=16, D=64) on 8 TRN2 cores.

Sharding: core c handles batch b = c//4 and head-group g = c%4 (4 heads,
256 embed cols). No cross-core communication; host slices inputs (pre-
transposed and pre-cast to bf16) and gathers/normalizes outputs.

Per-core device program (bf16 matmuls, fp32 PSUM accumulation):
  - projections: qhT[c, s] = sum_e wq[e, c] qT[e, s] (c on partitions), so
    attention needs no on-chip transposes; K-bias dropped (softmax-invariant),
    V-bias applied on host (distributes through softmax).
  - attention processes head PAIRS: the two scores matmuls run concurrently
    on disjoint PE row groups (K=64 each) into one [128, 1024] PSUM tile;
    one ScalarE exp (scale=1/8 fused) covers both heads per (iq, jt).
  - vh carries a ones column (m=64), so the out-stage accumulates the
    softmax denominator in PSUM row 64; host divides.
"""

import sys

sys.path.insert(0, "/opt/trn_rl_repo")

import numpy as np
import ml_dtypes

import concourse.bass as bass  # noqa: F401
import concourse.mybir as mybir
from concourse import bacc
from concourse.tile import TileContext

B, S, E = 2, 2048, 1024
H, D = 16, 64
HPC = 4  # heads per core
COLS = HPC * D  # 256
P = 128
F32 = mybir.dt.float32
BF16 = mybir.dt.bfloat16
ET = E // P  # 8 e-tiles
JT = S // P  # 16 j-tiles
NB = 512
NIQ = S // NB  # 4 i-quarters

_CACHED = {}


def build():
    nc = bacc.Bacc("TRN2", target_bir_lowering=False, debug=False)
    qT = nc.dram_tensor("qT", [E, S], BF16, kind="ExternalInput")
    kT = nc.dram_tensor("kT", [E, S], BF16, kind="ExternalInput")
    vT = nc.dram_tensor("vT", [E, S], BF16, kind="ExternalInput")
    wq = nc.dram_tensor("wq", [E, COLS], BF16, kind="ExternalInput")
    wk = nc.dram_tensor("wk", [E, COLS], BF16, kind="ExternalInput")
    wv = nc.dram_tensor("wv", [E, COLS], BF16, kind="ExternalInput")
    bq = nc.dram_tensor("bq", [P, 2], F32, kind="ExternalInput")
    # out_raw[:, (h*NIQ+iq)*NB : ...]: rows 0-63 numerator (d), row 64 denom
    out_raw = nc.dram_tensor("out_raw", [65, HPC * S], F32,
                             kind="ExternalOutput")  # [65, 8192]

    with TileContext(nc) as tc:
        with (
            tc.tile_pool(name="wp", bufs=1) as wp,
            tc.tile_pool(name="xq", bufs=ET) as xq,
            tc.tile_pool(name="xk", bufs=ET) as xk,
            tc.tile_pool(name="xv", bufs=ET) as xv,
            tc.tile_pool(name="hp", bufs=1) as hp,
            tc.tile_pool(name="pe", bufs=3) as pe,
            tc.tile_pool(name="psA", bufs=2, space="PSUM") as psA,
            tc.tile_pool(name="psO", bufs=4, space="PSUM") as psO,
        ):
            # --- weights + bias (tiny, first so projections never stall) ---
            wq_b = wp.tile([P, ET, COLS], BF16)
            wk_b = wp.tile([P, ET, COLS], BF16)
            wv_b = wp.tile([P, ET, COLS], BF16)
            nc.sync.dma_start(wq_b, wq.rearrange("(t p) c -> p t c", p=P))
            nc.sync.dma_start(wk_b, wk.rearrange("(t p) c -> p t c", p=P))
            nc.sync.dma_start(wv_b, wv.rearrange("(t p) c -> p t c", p=P))
            bq_t = wp.tile([P, 2], F32)
            nc.sync.dma_start(bq_t, bq[:, :])

            # --- activations, q/k interleaved first, v last ---
            def load_x(pool, dram, tag):
                tiles = []
                for et in range(ET):
                    t = pool.tile([P, S], BF16, tag=tag, name=f"{tag}{et}")
                    tiles.append(t)
                return tiles

            qx = load_x(xq, qT, "qx")
            kx = load_x(xk, kT, "kx")
            vx = load_x(xv, vT, "vx")
            for et in range(ET):
                nc.sync.dma_start(kx[et], kT[et * P : (et + 1) * P, :])
            for et in range(ET):
                nc.sync.dma_start(qx[et], qT[et * P : (et + 1) * P, :])
            for et in range(ET):
                nc.sync.dma_start(vx[et], vT[et * P : (et + 1) * P, :])

            # --- resident head tensors ---
            qhT = hp.tile([P, 2, S], BF16)  # [2 heads x 64 d, chunk, s]
            khT = hp.tile([P, 2, S], BF16)
            vh_aug = hp.tile([P, JT, HPC * 65], BF16)
            out_sb = hp.tile([P, H // HPC * NIQ * 2, NB], F32)  # [65used, 16, 512]
            nc.vector.memset(vh_aug, 1.0)

            # --- K/Q projections -> transposed head layout [c, s] ---
            # 4 PSUM slots accumulate in parallel (et outer, sb inner), so
            # consecutive matmuls hit different banks and pipeline at the
            # N-cycle rate instead of serializing on LDWEIGHTS.
            for name, w_b, dst, bias in (
                ("k", wk_b, khT, None),
                ("q", wq_b, qhT, bq_t),
            ):
                x = qx if name == "q" else kx
                for ch in range(2):
                    pss = [
                        psO.tile([P, NB], F32, tag="o", name=f"ps_{name}{ch}{sb}")
                        for sb in range(S // NB)
                    ]
                    for et in range(ET):
                        for sb in range(S // NB):
                            nc.tensor.matmul(
                                pss[sb],
                                w_b[:, et, ch * P : (ch + 1) * P],
                                x[et][:, sb * NB : (sb + 1) * NB],
                                start=(et == 0),
                                stop=(et == ET - 1),
                            )
                    for sb in range(S // NB):
                        if bias is not None:
                            nc.vector.tensor_scalar_add(
                                dst[:, ch, sb * NB : (sb + 1) * NB],
                                pss[sb],
                                bias[:, ch : ch + 1],
                            )
                        else:
                            nc.vector.tensor_copy(
                                dst[:, ch, sb * NB : (sb + 1) * NB], pss[sb]
                            )

            # --- V projection -> natural [s, c] layout (no bias) ---
            for sc4 in range(JT // 4):
                pss = [
                    psO.tile([P, NB], F32, tag="o", name=f"ps_v{j}") for j in range(4)
                ]
                for et in range(ET):
                    for j in range(4):
                        sc = sc4 * 4 + j
                        nc.tensor.matmul(
                            pss[j][:, :COLS],
                            vx[et][:, sc * P : (sc + 1) * P],
                            wv_b[:, et, :],
                            start=(et == 0),
                            stop=(et == ET - 1),
                        )
                for j in range(4):
                    sc = sc4 * 4 + j
                    nc.vector.tensor_copy(
                        vh_aug[:, sc].rearrange("p (h x) -> p h x", x=65)[:, :, :D],
                        pss[j][:, :COLS].rearrange("p (h x) -> p h x", x=D),
                    )

            # --- attention, head pairs ---
            for pr in range(2):  # heads (2pr, 2pr+1) = chunk pr
                for iq in range(NIQ):
                    op0 = psO.tile([P, NB], F32, tag="o", name="op0")
                    op1 = psO.tile([P, NB], F32, tag="o", name="op1")
                    for jt in range(JT):
                        sps = psA.tile([P, 1024], F32, tag="s", name="sps")
                        for hh in range(2):  # row-group packed, concurrent
                            r0 = hh * D
                            nc.tensor.matmul(
                                sps[:, hh * NB : (hh + 1) * NB],
                                khT[r0 : r0 + D, pr, jt * P : (jt + 1) * P],
                                qhT[r0 : r0 + D, pr, iq * NB : (iq + 1) * NB],
                                start=True,
                                stop=True,
                            )
                        expT = pe.tile([P, 1024], BF16, tag="e", name="expT")
                        nc.scalar.activation(
                            expT, sps, mybir.ActivationFunctionType.Exp, scale=0.125
                        )
                        for hh, op in ((0, op0), (1, op1)):
                            h = 2 * pr + hh
                            nc.tensor.matmul(
                                op[:65, :],
                                vh_aug[:, jt, h * 65 : (h + 1) * 65],
                                expT[:, hh * NB : (hh + 1) * NB],
                                start=(jt == 0),
                                stop=(jt == JT - 1),
                            )
                    for hh, op in ((0, op0), (1, op1)):
                        h = 2 * pr + hh
                        r = h * NIQ + iq
                        nc.vector.tensor_copy(out_sb[:65, r, :], op[:65, :])
                        nc.sync.dma_start(
                            out_raw[:, r * NB : (r + 1) * NB], out_sb[:65, r, :]
                        )
    nc.finalize()
    return nc


def _prep_in_maps(q, k, v, wq, bq, wk, bk, wv, bv):
    bf = ml_dtypes.bfloat16
    q, k, v = (np.asarray(x, np.float32) for x in (q, k, v))
    wqb, wkb, wvb = (np.asarray(x, bf) for x in (wq, wk, wv))
    bq = np.asarray(bq, np.float32)
    qT = [np.ascontiguousarray(q[b].T.astype(bf)) for b in range(B)]
    kT = [np.ascontiguousarray(k[b].T.astype(bf)) for b in range(B)]
    vT = [np.ascontiguousarray(v[b].T.astype(bf)) for b in range(B)]
    in_maps = []
    for c in range(8):
        b, g = divmod(c, 4)
        cs = slice(g * COLS, (g + 1) * COLS)
        in_maps.append(
            {
                "qT": qT[b],
                "kT": kT[b],
                "vT": vT[b],
                "wq": np.ascontiguousarray(wqb[:, cs]),
                "wk": np.ascontiguousarray(wkb[:, cs]),
                "wv": np.ascontiguousarray(wvb[:, cs]),
                "bq": np.ascontiguousarray(bq[cs].reshape(2, P).T),
            }
        )
    return in_maps


def _make_runner(nc, n_cores=8):
    """Persistent jitted shard_map runner over the prebuilt Bass module."""
    import jax
    from jax.experimental.shard_map import shard_map
    from jax.sharding import Mesh, NamedSharding, PartitionSpec
    from concourse import bass2jax

    bass2jax.install_neuronx_cc_hook()

    in_names, out_names, out_avals, zero_outs = [], [], [], []
    for alloc in nc.m.functions[0].allocations:
        if not isinstance(alloc, mybir.MemoryLocationSet):
            continue
        name = alloc.memorylocations[0].name
        if alloc.kind == "ExternalInput":
            in_names.append(name)
        elif alloc.kind == "ExternalOutput":
            shape = tuple(alloc.tensor_shape)
            dtype = mybir.dt.np(alloc.dtype)
            out_avals.append(jax.core.ShapedArray(shape, dtype))
            zero_outs.append(np.zeros((n_cores * shape[0], *shape[1:]), dtype))
            out_names.append(name)
    pid_name = nc.partition_id_tensor.name if nc.partition_id_tensor else None
    if pid_name is not None:
        in_names = [n for n in in_names if n != pid_name]
    n_params = len(in_names)
    all_names = in_names + out_names + ([pid_name] if pid_name else [])

    def _body(*args):
        operands = list(args)
        if pid_name is not None:
            operands.append(bass2jax.partition_id_tensor())
        outs = bass2jax._bass_exec_p.bind(
            *operands,
            out_avals=tuple(out_avals),
            in_names=tuple(all_names),
            out_names=tuple(out_names),
            lowering_input_output_aliases=(),
            sim_require_finite=True,
            sim_require_nnan=True,
            nc=nc,
        )
        return tuple(outs)

    devices = jax.devices()[:n_cores]
    mesh = Mesh(np.asarray(devices), ("core",))
    nio = n_params + len(out_names)
    sharded = jax.jit(
        shard_map(
            _body,
            mesh=mesh,
            in_specs=(PartitionSpec("core"),) * nio,
            out_specs=(PartitionSpec("core"),) * len(out_names),
            check_rep=False,
        ),
        keep_unused=True,
    )
    row_sharding = NamedSharding(mesh, PartitionSpec("core"))
    zeros_dev = [jax.device_put(z, row_sharding) for z in zero_outs]

    def run(in_maps):
        concat_in = [
            np.concatenate([np.asarray(m[name]) for m in in_maps], axis=0)
            for name in in_names
        ]
        out_arrs = sharded(*concat_in, *zeros_dev)
        return [
            {
                name: np.asarray(out_arrs[i]).reshape(n_cores, *out_avals[i].shape)[c]
                for i, name in enumerate(out_names)
            }
            for c in range(n_cores)
        ]

    run.sharded = sharded
    run.in_names = in_names
    run.zeros_dev = zeros_dev
    run.row_sharding = row_sharding
    return run


def get_runner():
    if "run" not in _CACHED:
        _CACHED["nc"] = build()
        _CACHED["run"] = _make_runner(_CACHED["nc"])
    return _CACHED["run"]


def kernel(q, k, v, wq, bq, wk, bk, wv, bv):
    run = get_runner()
    in_maps = _prep_in_maps(q, k, v, wq, bq, wk, bk, wv, bv)
    results = run(in_maps)

    bv = np.asarray(bv, np.float32)
    out = np.empty((B, S, E), np.float32)
    for c in range(8):
        b, g = divmod(c, 4)
        raw = results[c]["out_raw"]  # [65, 8192]
        num = raw[:64].reshape(64, HPC, S)  # [d, h, i] (NIQ*NB = S)
        den = raw[64].reshape(HPC, S)
        for h in range(HPC):
            col0 = g * COLS + h * D
            o = num[:, h, :] / den[h][None, :]
            out[b, :, col0 : col0 + D] = o.T + bv[col0 : col0 + D][None, :]
    return out
